# revision 21
# baseline (speedup 1.0000x reference)
"""Trainium2 Bass kernel for the per-cell star-graph GAT encoder.

Math: the reference returns only the anchor-node (node 0) output of a 1-layer
GAT over a (T+1)-node graph per cell. The anchor's adjacency row contains only
the star edges (anchor self-loop + all valid transcripts), so the kNN graph is
dead code for the output. With h_anchor = 0 the output reduces, per cell, to

    s_j    = tx_feat_j . (W_gat @ a_src)            (per transcript scalar)
    l_j    = leaky_relu(s_j, 0.2)  (+ -1e30 where invalid)
    e_j    = exp(l_j)        den = 1 + sum_j e_j    (anchor logit is 0)
    pooled = sum_j e_j tx_feat_j / den              (35-dim)
    out    = pooled @ W_gat + b_gat, zeroed where cell inactive

where tx_feat_j = [rel_xy (2) | gene_emb[id_j] (32) | qv (1)].

Sharding: data-parallel over the 1024 = B*Q cells, 128 cells per NeuronCore.
Per core, cells sit on SBUF partitions; the gene table (with the per-gene
scalar gs = gene_emb @ wa[2:34] prepended as column 0, padded to 256B rows)
is gathered from DRAM by dma_gather in tapering t-chunks of 4 quarter ops
each, spread over the 4 SWDGE queues (pattern 1,2,3,0: three async queues
dispatch instantly, queue 0 holds the engine while all four generate
descriptors concurrently), pipelined against the DVE/ACT compute.

The gather phase is Q7 descriptor-generation bound (~8.4 ns/idx/queue).
x/qv ride interleaved as [x0,x1,qv] triplets so every pooled-x/qv pass is one
mult + one reduce; the pooled vector is laid out [px0,px1,pqv,genes] with
W_gat rows permuted to match. Chunks 0..2 close pooled_p early (their gene
pools + t<TM2 x/qv) so the first bf16 half-matmul fires mid-gather; chunks
3..4 feed pooled_l with a hand-ordered minimal DVE tail. Output is bf16,
widened on host.
"""

import numpy as np

import concourse.bacc as bacc
import concourse.mybir as mybir
import concourse.tile as tile
from concourse.bass_utils import run_bass_kernel_spmd
from concourse.tile import add_dep_helper

F32 = mybir.dt.float32
BF16 = mybir.dt.bfloat16
I16 = mybir.dt.int16
AX = mybir.AxisListType
OP = mybir.AluOpType
AF = mybir.ActivationFunctionType

N_CORES = 8
B, Q, T = 4, 256, 128
CELLS = (B * Q) // N_CORES  # 128 cells per core
GENE_VOCAB = 20000
GENE_DIM = 32
F = 64  # padded table row: [gs | gene_emb(32) | zeros], 256B for dma_gather
IN_DIM = 35
D = 256
NEG_SLOPE = 0.2
CHUNK_T = (48, 32, 24, 16, 8)  # big early, tiny tail round
NCH = len(CHUNK_T)
SINGLE_PACKET = False
assert sum(CHUNK_T) == T
TM2 = sum(CHUNK_T[:3])  # 104: t < TM2 pooled into pooled_p's x/qv cols

# packed input column layout (f32 columns)
C_XQ = 0                  # [x0|x1|qv] per t: T*3
C_MADD = C_XQ + T * 3     # 384
C_CENT = C_MADD + T       # 512: [c0, c1, 0]
C_ACT = C_CENT + 3        # 515
C_WAT = C_ACT + 1         # 516: [wa0,wa1,wa34] tiled, T*3
C_BG = C_WAT + T * 3      # 900: b_gat, 256
C_ID = C_BG + 256         # 1156: identity 128
C_WR = C_ID + 128         # 1284: wrhs bf16 [128, 256] as 128 f32
C_IDX = C_WR + 128        # 1412: idx int16 [128, 1024] as 512 f32
NB = C_IDX + (CELLS * T) // 32  # 1924

_CACHE = {}


def build_program():
    nc = bacc.Bacc("TRN2", target_bir_lowering=False, debug=False,
                   num_devices=N_CORES, num_swdge_queues=4)

    big_d = nc.dram_tensor("big_in", [CELLS, NB], F32, kind="ExternalInput")
    table_d = nc.dram_tensor("table_in", [GENE_VOCAB, F], F32, kind="ExternalInput")
    out_d = nc.dram_tensor("out", [CELLS, D], BF16, kind="ExternalOutput")

    from concourse import library_config

    with tile.TileContext(nc) as tc:
        with (
            tc.tile_pool(name="single", bufs=1) as single,
            tc.tile_pool(name="gpool", bufs=NCH) as gpool,
            tc.tile_pool(name="work", bufs=2) as work,
            tc.tile_pool(name="stats", bufs=NCH + 2) as stats,
            tc.tile_pool(name="psum", bufs=2, space="PSUM") as psum,
        ):
            # the mlp-library Q7 IRAM reload (~11us) gates the first gather;
            # it must be the first thing on the gpsimd stream
            nc.gpsimd.load_library(library_config.mlp)

            # one packed DMA for every dense input (sync HWDGE ring)
            big = single.tile([CELLS, NB], F32)
            nc.sync.dma_start(out=big[:], in_=big_d.ap())
            xq_sb = big[:, C_XQ:C_XQ + T * 3]
            madd_sb = big[:, C_MADD:C_MADD + T]
            cent_sb = big[:, C_CENT:C_CENT + 3]
            act_sb = big[:, C_ACT:C_ACT + 1]
            wat_sb = big[:, C_WAT:C_WAT + T * 3]
            bg_sb = big[:, C_BG:C_BG + 256]
            ident = big[:, C_ID:C_ID + 128]
            wrhs_sb = big[:IN_DIM, C_WR:C_WR + 128].bitcast(BF16)  # [35, 256]
            idx_sb = big[:, C_IDX:NB].bitcast(I16)  # [128, 1024] int16

            # --- gather pipeline (critical path) ---
            # queue pattern [1,2,3,0] per chunk: the three async queues
            # dispatch instantly, then the sync queue-0 op holds the engine
            # while all four queues' desc-gen runs concurrently. NOTE: Tile
            # has 8 SWDGE sem lanes, so op k serializes behind op k-8's DRAIN
            # completion -- keep rounds >= 512 idxs/op late or they stagger.
            g_t = []
            t_base = 0
            for j in range(NCH):
                tcnt = CHUNK_T[j]
                tq = tcnt // 4  # transcripts per quarter-gather
                gj = gpool.tile([CELLS, max(CHUNK_T) * F], F32, tag="g")
                g3w = gj[:].rearrange("p (t f) -> p t f", t=max(CHUNK_T), f=F)
                for sub, queue in enumerate((1, 2, 3, 0)):
                    c0 = (CELLS * (t_base + tq * sub)) // 16
                    nc.gpsimd.dma_gather(
                        out_ap=g3w[:, tq * sub:tq * (sub + 1), :],
                        in_ap=table_d.ap(),
                        idxs_ap=idx_sb[:, c0:c0 + (CELLS * tq) // 16],
                        num_idxs=CELLS * tq,
                        num_idxs_reg=CELLS * tq,
                        elem_size=F,
                        single_packet=SINGLE_PACKET,
                        queue_num=queue,
                    )
                g_t.append(gj)
                t_base += tcnt

            # b_gat * active, ready off the critical path for the epilogue
            bact = single.tile([CELLS, D], F32)
            nc.vector.tensor_scalar_mul(bact[:], bg_sb, act_sb)

            # --- s_pre = (xq . wa3) - cent.wa01 + madd  (no gene term) ---
            xw = work.tile([CELLS, T * 3], F32)
            nc.vector.tensor_tensor(out=xw[:], in0=xq_sb, in1=wat_sb, op=OP.mult)
            term = single.tile([CELLS, T], F32)
            nc.vector.tensor_reduce(
                out=term[:],
                in_=xw[:].rearrange("p (t c) -> p t c", t=T, c=3),
                axis=AX.X, op=OP.add)
            cw = stats.tile([CELLS, 2], F32, tag="cw")
            nc.vector.tensor_tensor(out=cw[:], in0=cent_sb[:, 0:2],
                                    in1=wat_sb[:, 0:2], op=OP.mult)
            c01 = stats.tile([CELLS, 1], F32, tag="c01")
            nc.vector.tensor_reduce(out=c01[:], in_=cw[:], axis=AX.X, op=OP.add)
            negc01 = stats.tile([CELLS, 1], F32, tag="negc01")
            nc.vector.tensor_scalar_mul(negc01[:], c01[:], -1.0)
            spre0 = work.tile([CELLS, T], F32, tag="spre0")
            nc.scalar.activation(out=spre0[:], in_=term[:], func=AF.Identity,
                                 bias=negc01[:], scale=1.0)
            spre = single.tile([CELLS, T], F32)
            nc.vector.tensor_tensor(out=spre[:], in0=spre0[:], in1=madd_sb, op=OP.add)

            # --- chunks 0..2: s -> lrelu -> exp -> gene pool; accumulate into
            # pooled_p (gene cols 3:35) + batched x/qv pool over t<TM2, then
            # the early bf16 half-matmul (PSUM start=True) fires mid-gather.
            e_all = single.tile([CELLS, T], F32, name="e_all", tag="e_all")
            pooled_p = single.tile([CELLS, IN_DIM], F32)
            pooled_l = single.tile([CELLS, IN_DIM], F32)
            acc_es = None
            acc_pg = None
            prev_inst = None

            def chain(inst):
                nonlocal prev_inst
                if prev_inst is not None:
                    add_dep_helper(inst.ins, prev_inst.ins, False, "dve order")
                prev_inst = inst
                return inst

            def s_l_exp(j, t0, tcnt):
                cj = slice(t0, t0 + tcnt)
                g3 = g_t[j][:].rearrange("p (t f) -> p t f", t=max(CHUNK_T),
                                         f=F)[:, :tcnt, :]
                s_j = work.tile([CELLS, tcnt], F32, tag="s")
                chain(nc.vector.tensor_tensor(out=s_j[:], in0=spre[:, cj],
                                              in1=g3[:, :, 0:1], op=OP.add))
                l_j = work.tile([CELLS, tcnt], F32, tag="l")
                nc.vector.scalar_tensor_tensor(out=l_j[:], in0=s_j[:],
                                               scalar=NEG_SLOPE, in1=s_j[:],
                                               op0=OP.mult, op1=OP.max)
                esum_j = stats.tile([CELLS, 1], F32, tag="esum")
                nc.scalar.activation(out=e_all[:, cj], in_=l_j[:], func=AF.Exp,
                                     accum_out=esum_j[:])
                return cj, g3, esum_j

            def eg_mult(cj, g3, tcnt):
                eg_j = work.tile([CELLS, tcnt * GENE_DIM], F32, tag="eg")
                chain(nc.vector.tensor_tensor(
                    out=eg_j[:], in0=g3[:, :, 1:1 + GENE_DIM],
                    in1=e_all[:, cj].to_broadcast([CELLS, tcnt, GENE_DIM]),
                    op=OP.mult))
                return eg_j

            def pg_reduce(eg_ap, tcnt, out_ap):
                return chain(nc.vector.tensor_reduce(
                    out=out_ap,
                    in_=eg_ap.rearrange("p (t f) -> p f t", t=tcnt, f=GENE_DIM),
                    axis=AX.X, op=OP.add))

            t_base = 0
            for j in range(3):
                tcnt = CHUNK_T[j]
                cj, g3, esum_j = s_l_exp(j, t_base, tcnt)
                eg_j = eg_mult(cj, g3, tcnt)
                if j < 2:
                    pg_j = stats.tile([CELLS, GENE_DIM], F32, tag="pg")
                    pg_reduce(eg_j[:], tcnt, pg_j[:])
                else:
                    pg_reduce(eg_j[:], tcnt, pooled_l[:, 3:3 + GENE_DIM])
                    pg_j = None
                if acc_es is None:
                    acc_es, acc_pg = esum_j, pg_j
                elif j < 2:
                    new_es = stats.tile([CELLS, 1], F32, tag="aes")
                    nc.vector.tensor_tensor(out=new_es[:], in0=acc_es[:],
                                            in1=esum_j[:], op=OP.add)
                    new_pg = stats.tile([CELLS, GENE_DIM], F32, tag="apg")
                    nc.vector.tensor_tensor(out=new_pg[:], in0=acc_pg[:],
                                            in1=pg_j[:], op=OP.add)
                    acc_es, acc_pg = new_es, new_pg
                else:
                    # chunk2 gene went to a staging slot; pooled_p gene =
                    # acc(0,1) + chunk2 (reuses pooled_l's gene cols as tmp)
                    new_es = stats.tile([CELLS, 1], F32, tag="aes")
                    nc.vector.tensor_scalar(new_es[:], acc_es[:], esum_j[:],
                                            1.0, OP.add, OP.add)
                    acc_es = new_es
                    chain(nc.vector.tensor_tensor(
                        out=pooled_p[:, 3:3 + GENE_DIM], in0=acc_pg[:],
                        in1=pooled_l[:, 3:3 + GENE_DIM], op=OP.add))
                t_base += tcnt

            # batched x/qv pool over t < TM2 into pooled_p[0:3]
            bxq = work.tile([CELLS, TM2 * 3], F32, tag="exall")
            chain(nc.vector.tensor_tensor(
                out=bxq[:], in0=xq_sb[:, :TM2 * 3],
                in1=e_all[:, :TM2].to_broadcast([CELLS, TM2, 3]), op=OP.mult))
            chain(nc.vector.tensor_reduce(
                out=pooled_p[:, 0:3],
                in_=bxq[:].rearrange("p (t c) -> p c t", t=TM2, c=3),
                axis=AX.X, op=OP.add))

            # early half-matmul: pooled_p complete; fires mid-gather
            psum_t1 = psum.tile([128, 128], F32, tag="pt")
            nc.tensor.transpose(out=psum_t1[:IN_DIM, :CELLS],
                                in_=pooled_p[:], identity=ident)
            lhsT1 = single.tile([128, CELLS], BF16)
            nc.scalar.copy(lhsT1[:IN_DIM, :], psum_t1[:IN_DIM, :CELLS])
            out_ps = psum.tile([128, D], F32, tag="out")
            nc.tensor.matmul(out=out_ps[:], lhsT=lhsT1[:IN_DIM, :],
                             rhs=wrhs_sb, start=True, stop=False)

            # --- chunk 3 (16t): s/l/exp + gene pool, hidden under the last
            # two gather rounds; its gene pool lands in pooled_l via g3s.
            t3 = TM2
            cj3, g33, es3 = s_l_exp(3, t3, CHUNK_T[3])
            es_p = stats.tile([CELLS, 1], F32, tag="aes")
            nc.vector.tensor_tensor(out=es_p[:], in0=acc_es[:], in1=es3[:],
                                    op=OP.add)
            eg3 = eg_mult(cj3, g33, CHUNK_T[3])
            lh = CHUNK_T[3] // 2
            pg3a = stats.tile([CELLS, GENE_DIM], F32, tag="pg")
            chain(nc.vector.tensor_reduce(
                out=pg3a[:],
                in_=eg3[:, :lh * GENE_DIM].rearrange("p (t f) -> p f t", t=lh,
                                                     f=GENE_DIM),
                axis=AX.X, op=OP.add))
            pg3b = stats.tile([CELLS, GENE_DIM], F32, tag="pg")
            chain(nc.vector.tensor_reduce(
                out=pg3b[:],
                in_=eg3[:, lh * GENE_DIM:].rearrange("p (t f) -> p f t",
                                                     t=CHUNK_T[3] - lh,
                                                     f=GENE_DIM),
                axis=AX.X, op=OP.add))
            g3s = stats.tile([CELLS, GENE_DIM], F32, tag="apg")
            chain(nc.vector.tensor_tensor(out=g3s[:], in0=pg3a[:],
                                          in1=pg3b[:], op=OP.add))

            # --- chunk 4 (8t) minimal tail ---
            t4 = TM2 + CHUNK_T[3]
            TL = CHUNK_T[4]
            cj4, g34, es_t = s_l_exp(4, t4, TL)
            eg_t = eg_mult(cj4, g34, TL)
            pg4 = stats.tile([CELLS, GENE_DIM], F32, tag="pg")
            pg_reduce(eg_t[:], TL, pg4[:])
            chain(nc.vector.tensor_tensor(out=pooled_l[:, 3:3 + GENE_DIM],
                                          in0=g3s[:], in1=pg4[:], op=OP.add))
            den = stats.tile([CELLS, 1], F32, tag="den")
            chain(nc.vector.tensor_tensor(out=den[:], in0=es_p[:],
                                          in1=es_t[:], op=OP.add))
            cs = stats.tile([CELLS, 3], F32, tag="cs")
            chain(nc.vector.scalar_tensor_tensor(out=cs[:], in0=cent_sb,
                                                 scalar=den[:], in1=cent_sb,
                                                 op0=OP.mult, op1=OP.subtract))
            exq = stats.tile([CELLS, (T - TM2) * 3], F32, tag="exl")
            chain(nc.vector.tensor_tensor(
                out=exq[:], in0=xq_sb[:, TM2 * 3:],
                in1=e_all[:, TM2:].to_broadcast([CELLS, T - TM2, 3]),
                op=OP.mult))
            pxq = stats.tile([CELLS, 3], F32, tag="pxl")
            chain(nc.vector.tensor_reduce(
                out=pxq[:],
                in_=exq[:].rearrange("p (t c) -> p c t", t=T - TM2, c=3),
                axis=AX.X, op=OP.add))
            # pooled_l[0:3] = pxq - cent3*(den-1); qv col's cent is 0
            chain(nc.vector.tensor_tensor(out=pooled_l[:, 0:3], in0=pxq[:],
                                          in1=cs[:], op=OP.subtract))
            # off the critical chain: softmax scale for the final epilogue
            rec = stats.tile([CELLS, 1], F32, tag="rec")
            chain(nc.vector.reciprocal(rec[:], den[:]))
            ra = stats.tile([CELLS, 1], F32, tag="ra")
            chain(nc.vector.tensor_scalar_mul(ra[:], rec[:], act_sb))

            psum_t2 = psum.tile([128, 128], F32, tag="pt")
            nc.tensor.transpose(out=psum_t2[:IN_DIM, :CELLS], in_=pooled_l[:],
                                identity=ident)
            lhsT2 = single.tile([128, CELLS], BF16)
            nc.scalar.copy(lhsT2[:IN_DIM, :], psum_t2[:IN_DIM, :CELLS])
            nc.tensor.matmul(out=out_ps[:], lhsT=lhsT2[:IN_DIM, :],
                             rhs=wrhs_sb, start=False, stop=True)
            out_sb = work.tile([CELLS, D], BF16, tag="outs")
            nc.vector.scalar_tensor_tensor(out=out_sb[:], in0=out_ps[:],
                                           scalar=ra[:], in1=bact[:],
                                           op0=OP.mult, op1=OP.add)
            nc.sync.dma_start(out=out_d.ap(), in_=out_sb[:])

    nc.compile()
    return nc


def host_prep(omics_x, centroids, omics_gene_ids, omics_qv, omics_valid_mask,
              query_valid_mask, gene_emb, W_gat, a_src, a_dst, b_gat):
    import ml_dtypes
    f32 = np.float32
    wa = (W_gat.astype(np.float64) @ a_src.astype(np.float64)).astype(f32)  # [35]
    gs = (gene_emb.astype(f32) @ wa[2:2 + GENE_DIM]).astype(f32)  # [VOCAB]
    table = np.zeros((GENE_VOCAB, F), f32)  # [VOCAB, 64] (256B rows)
    table[:, 0] = gs
    table[:, 1:1 + GENE_DIM] = gene_emb.astype(f32)

    NC_TOT = B * Q
    x = omics_x.astype(f32).reshape(NC_TOT, T, 2)
    qv = omics_qv.astype(f32).reshape(NC_TOT, T, 1)
    xq = np.concatenate([x, qv], axis=2).reshape(NC_TOT, T * 3)
    ids = omics_gene_ids.astype(np.int16).reshape(NC_TOT, T)
    cent = centroids.astype(f32).reshape(NC_TOT, 2)
    validf = omics_valid_mask.reshape(NC_TOT, T).astype(f32)
    madd = (validf - 1.0) * f32(1e30)
    active = (query_valid_mask.reshape(NC_TOT).astype(bool)
              & omics_valid_mask.reshape(NC_TOT, T).astype(bool).any(-1))
    active = active.astype(f32)

    # pooled column order is [x0, x1, qv, genes]: permute W rows to match
    perm = [0, 1, 34] + list(range(2, 34))
    wrhs_pad = np.zeros((128, D), f32)
    wrhs_pad[:IN_DIM] = W_gat.astype(f32)[perm]
    wrhs_bf = np.ascontiguousarray(
        wrhs_pad.astype(ml_dtypes.bfloat16)).view(np.uint16)  # [128, 256] u16
    wa3 = wa[[0, 1, 34]]

    in_maps = []
    for c in range(N_CORES):
        sl = slice(c * CELLS, (c + 1) * CELLS)
        big = np.zeros((CELLS, NB), f32)
        big[:, C_XQ:C_XQ + T * 3] = xq[sl]
        big[:, C_MADD:C_MADD + T] = madd[sl]
        big[:, C_CENT:C_CENT + 2] = cent[sl]
        big[:, C_ACT] = active[sl]
        big[:, C_WAT:C_WAT + T * 3] = np.tile(wa3, T)[None, :]
        big[:, C_BG:C_BG + 256] = b_gat.astype(f32)[None, :]
        big[:, C_ID:C_ID + 128] = np.eye(128, dtype=f32)
        big[:, C_WR:C_WR + 128] = wrhs_bf.view(f32)
        # flat gather index i = t*CELLS + cell -> dst[cell, t]; wrapped
        # [128, T*CELLS/16] int16 (idx list tiled down the 8 core slabs)
        flat = ids[sl].T.reshape(-1)
        wrapped = np.ascontiguousarray(
            np.tile(flat.reshape(-1, 16).T, (8, 1)))  # [128, 1024] i16
        big[:, C_IDX:NB] = wrapped.reshape(128, -1).view(f32)
        in_maps.append({"big_in": np.ascontiguousarray(big), "table_in": table})
    return in_maps


def _get_program():
    # the program is fully parameter-independent: one compile, ever
    if "prog" not in _CACHE:
        _CACHE["prog"] = build_program()
    return _CACHE["prog"]


def kernel(omics_x, centroids, omics_gene_ids, omics_qv, omics_valid_mask,
           query_valid_mask, gene_emb, W_gat, a_src, a_dst, b_gat,
           trace=False):
    in_maps = host_prep(
        np.asarray(omics_x), np.asarray(centroids), np.asarray(omics_gene_ids),
        np.asarray(omics_qv), np.asarray(omics_valid_mask),
        np.asarray(query_valid_mask), np.asarray(gene_emb), np.asarray(W_gat),
        np.asarray(a_src), np.asarray(a_dst), np.asarray(b_gat))
    nc = _get_program()
    res = run_bass_kernel_spmd(nc, in_maps, core_ids=list(range(N_CORES)),
                               trace=trace)
    global LAST_RESULTS
    LAST_RESULTS = res
    outs = [np.asarray(res.results[c]["out"]).astype(np.float32)
            for c in range(N_CORES)]
    full = np.concatenate(outs, axis=0).reshape(B, Q, D)
    return full.astype(np.float32)
